# revision 2
# baseline (speedup 1.0000x reference)
"""Trainium2 Bass kernel for BaseSegHead (dynamic 1x1-conv seg logits).

Computes, for full inputs:
    qry_feats = in_feats @ qry_w.T + qry_b                  [1200, 32]
    key_map   = einsum('oc,bchw->bohw', key_w, feat_map) + key_b
    logits    = einsum('bnc,bchw->bnhw', qry_feats.reshape(4,300,32), key_map)
    out       = logits.reshape(1200, 160, 160)

Sharding: 8 cores = 4 batch images x 2 spatial (H) halves. Core c handles
batch b = c//2, rows h*80:(h+1)*80. Each core reads feat_map[b,:,rows,:],
its 300 queries, and writes a [300, 80*160] output shard -- no cross-core
communication and no duplicated feat_map reads.

Precision: matmul operands are shipped/produced as fp16 (full-rate on the
PE array; halves DMA bytes); accumulation stays fp32 in PSUM. The fp32
logits are rounded to fp16 for the output DMA and upcast on the host.

v2 layout (trace-driven): the kernel is HBM-bound (~358 GB/s/core cap,
14.4 MB of traffic => ~40us floor). All data DMAs ride the single sync
HW-DGE ring, enqueued in readiness order: consts, then 4 big feat
triggers (no waits -> the ring is never starved on the input side), then
12 output quarter-triggers that fire as their PSUM drains complete.
Output is staged in three full-row SBUF buffers [*, 12800] so each out
DMA moves an entire chunk quarter (6KB rows). PSUM drains are 2-bank
[*, 1024] copies alternating scalar/vector to halve instruction count
and amortize the per-instruction overhead.

TensorE array tiling: the key projection (M=32) runs 4-way column-tiled
into one PSUM bank per quad of hw-tiles; one bias-activation drains four
tiles. The main einsum (K=32) runs 4-way row-tiled: hw-tile t keeps its
q and key_map operands on SBUF partitions 32*(t%4), so consecutive tiles
issue to distinct PE row-groups and overlap on the array.
"""

import os
import sys

sys.path.insert(0, "/opt/trn_rl_repo")
os.environ.setdefault("MYCRO_LOCAL_CACHE", "1")

import numpy as np

BATCH = 4
N_PER = 300
IN_DIM = 256
KEY_DIM = 32
FH = FW = 160
HHALF = FH // 2            # 80 rows per core
HW = HHALF * FW            # 12800 spatial positions per core
N_CORES = 8

MMN = 512                  # matmul moving free size (one fp32 PSUM bank)
N_T = HW // MMN            # 25 hw-tiles
N_QUADS = (N_T + 3) // 4   # 7 key quads (last holds a single tile)
N_PAIRS = N_T // 2         # 12 pairs in the main loop (+1 tail tile)
FSPLIT = 13 * MMN          # 6656: feat DMA column split (13 + 12 tiles)
# out-DMA quarters: col ranges per chunk (last one absorbs the tail tile)
QUARTERS = ((0, 3072), (3072, 6144), (6144, 9216), (9216, HW))
N_CHUNKS = ((0, 128), (128, 128), (256, 44))   # query-row chunks (300 rows)
CPACK_W = 728              # fp16: qry_wT (64) + in_featsT (600) + key_wT (64)

_CACHE = {}


def build_nc():
    import concourse.bass as bass
    import concourse.bacc as bacc
    import concourse.mybir as mybir
    from concourse import tile

    f32 = mybir.dt.float32
    f16 = mybir.dt.float16
    Ident = mybir.ActivationFunctionType.Identity

    nc = bacc.Bacc("TRN2", target_bir_lowering=False, debug=False)

    featA = nc.dram_tensor("featA", [128, HW], f16, kind="ExternalInput")
    featB = nc.dram_tensor("featB", [128, HW], f16, kind="ExternalInput")
    cpack = nc.dram_tensor("cpack", [128, CPACK_W], f16, kind="ExternalInput")
    bpack = nc.dram_tensor("bpack", [128, 2], f32, kind="ExternalInput")
    out = nc.dram_tensor("out", [128, 3 * HW], f16, kind="ExternalOutput")

    with tile.TileContext(nc) as tc:
        with (
            tc.tile_pool(name="const", bufs=1) as cpool,
            tc.tile_pool(name="fpool", bufs=1) as fpool,
            tc.tile_pool(name="opool", bufs=1) as opool,
            tc.tile_pool(name="kmap", bufs=1) as kpool,
            tc.tile_pool(name="ps_main", bufs=3, space=bass.MemorySpace.PSUM) as ps_main,
            tc.tile_pool(name="ps_small", bufs=2, space=bass.MemorySpace.PSUM) as ps_small,
        ):
            # --- DMA ring head: consts, then the four big feat loads -----
            ct = cpool.tile([128, CPACK_W], f16, name="ct")
            nc.sync.dma_start(ct[:], cpack[:])
            bt = cpool.tile([128, 2], f32, name="bt")
            nc.sync.dma_start(bt[:], bpack[:])
            qw = (ct[:, 0:32], ct[:, 32:64])
            inT = (ct[:, 64:364], ct[:, 364:664])
            kw = (ct[:, 664:696], ct[:, 696:728])
            qb = bt[:, 0:1]        # qry_b replicated in all four bands
            kb = bt[:, 1:2]        # key_b replicated in all four bands

            fa = fpool.tile([128, HW], f16, name="fa")
            fb = fpool.tile([128, HW], f16, name="fb")
            nc.sync.dma_start(fa[:, 0:FSPLIT], featA[:, 0:FSPLIT])
            nc.sync.dma_start(fb[:, 0:FSPLIT], featB[:, 0:FSPLIT])
            nc.sync.dma_start(fa[:, FSPLIT:HW], featA[:, FSPLIT:HW])
            nc.sync.dma_start(fb[:, FSPLIT:HW], featB[:, FSPLIT:HW])
            F = (fa, fb)

            # --- qry projection, 4-way column-tiled (4 band copies) -------
            qp = ps_small.tile([128, MMN], f32, name="qp", tag="kp")
            for b in range(4):
                for d in range(2):
                    nc.tensor.matmul(
                        qp[32 * b:32 * b + 32, 0:N_PER],
                        qw[d],
                        inT[d],
                        start=(d == 0),
                        stop=(d == 1),
                        tile_position=(0, 32 * b),
                    )
            q_sb = cpool.tile([128, N_PER], f16, name="q_sb")
            nc.scalar.activation(q_sb[:], qp[:, 0:N_PER], Ident, bias=qb)

            # --- key_map: 4-way column-tiled, banded layout ---------------
            # hw-tile t lives on SBUF partitions 32*(t%4), columns
            # (t//4)*512; one [128,512] PSUM bank holds a whole quad and is
            # drained by a single bias-activation.
            key_map = kpool.tile([128, N_QUADS * MMN], f16, name="key_map")

            def key_quad(k):
                kp = ps_small.tile([128, MMN], f32, name=f"kp_{k}", tag="kp")
                nb = min(4, N_T - 4 * k)
                for b in range(nb):
                    col0 = (4 * k + b) * MMN
                    for d in range(2):
                        nc.tensor.matmul(
                            kp[32 * b:32 * b + 32, :],
                            kw[d],
                            F[d][:, col0:col0 + MMN],
                            start=(d == 0),
                            stop=(d == 1),
                            tile_position=(0, 32 * b),
                        )
                p = 32 * nb
                nc.scalar.activation(
                    key_map[0:p, k * MMN:(k + 1) * MMN], kp[0:p, :], Ident,
                    bias=kb[0:p, :],
                )

            # --- output row-buffers: one [*, 12800] tile per query chunk --
            OB = [
                opool.tile([128, HW], f16, name=f"ob_{j}")
                for j in range(3)
            ]

            # --- main einsum: 4-way row-tiled over band b = t%4 -----------
            # Pairs of hw-tiles share a 2-bank PSUM tile so one copy drains
            # both; drains alternate scalar/vector.
            cp = 0

            def drain(dst, src):
                nonlocal cp
                if cp % 2 == 0:
                    nc.scalar.copy(dst, src)
                else:
                    nc.vector.tensor_copy(dst, src)
                cp += 1

            def main_pair(p):
                t0 = 2 * p
                for (n0, m) in N_CHUNKS:
                    mp = ps_main.tile([128, 2 * MMN], f32, name=f"mp_{p}_{n0}", tag="mp")
                    for i, t in enumerate((t0, t0 + 1)):
                        b = t % 4
                        kcol = (t // 4) * MMN
                        nc.tensor.matmul(
                            mp[:m, i * MMN:(i + 1) * MMN],
                            q_sb[32 * b:32 * b + 32, n0:n0 + m],
                            key_map[32 * b:32 * b + 32, kcol:kcol + MMN],
                            tile_position=(32 * b, 0),
                        )
                    drain(OB[(n0 > 0) + (n0 > 128)][:m, t0 * MMN:(t0 + 2) * MMN], mp[:m, :])

            def main_tail():
                # hw-tile 24: band 0, key_map quad 6
                for j, (n0, m) in enumerate(N_CHUNKS):
                    mp = ps_small.tile([128, MMN], f32, name=f"mt_{n0}", tag="kp")
                    nc.tensor.matmul(
                        mp[:m, :],
                        q_sb[0:32, n0:n0 + m],
                        key_map[0:32, 6 * MMN:7 * MMN],
                        tile_position=(0, 0),
                    )
                    drain(OB[j][:m, 24 * MMN:25 * MMN], mp[:m, :])

            # Interleave: each key quad feeds two main pairs.
            for k in range(6):
                key_quad(k)
                main_pair(2 * k)
                main_pair(2 * k + 1)
            key_quad(6)
            main_tail()

            # --- out DMAs: quarter-granularity, readiness order ----------
            for (c0, c1) in QUARTERS:
                for j, (n0, m) in enumerate(N_CHUNKS):
                    nc.sync.dma_start(
                        out[0:m, j * HW + c0:j * HW + c1], OB[j][0:m, c0:c1]
                    )

    nc.compile()
    return nc


def _get_nc():
    if "nc" not in _CACHE:
        _CACHE["nc"] = build_nc()
    return _CACHE["nc"]


def make_in_maps(in_feats, feat_map, qry_w, qry_b, key_b, key_w):
    qwT = qry_w.T.astype(np.float16)                          # [256, 32]
    kwT = key_w.T.astype(np.float16)                          # [256, 32]
    bpack = np.zeros((128, 2), np.float32)
    bpack[:, 0] = np.tile(qry_b, 4)
    bpack[:, 1] = np.tile(key_b, 4)
    in_maps = []
    for c in range(N_CORES):
        b, h = divmod(c, 2)
        ifT = in_feats[b * N_PER:(b + 1) * N_PER].T.astype(np.float16)
        cpack = np.zeros((128, CPACK_W), np.float16)
        cpack[:, 0:32] = qwT[0:128]
        cpack[:, 32:64] = qwT[128:256]
        cpack[:, 64:364] = ifT[0:128]
        cpack[:, 364:664] = ifT[128:256]
        cpack[:, 664:696] = kwT[0:128]
        cpack[:, 696:728] = kwT[128:256]
        feat16 = np.ascontiguousarray(
            feat_map[b, :, h * HHALF:(h + 1) * HHALF, :]
        ).reshape(IN_DIM, HW).astype(np.float16)
        in_maps.append({
            "featA": np.ascontiguousarray(feat16[0:128]),
            "featB": np.ascontiguousarray(feat16[128:256]),
            "cpack": cpack,
            "bpack": bpack,
        })
    return in_maps


def kernel(**inputs):
    in_feats = np.asarray(inputs["in_feats"], dtype=np.float32)
    feat_map = np.asarray(inputs["feat_map"], dtype=np.float32)
    qry_w = np.asarray(inputs["qry_w"], dtype=np.float32)
    qry_b = np.asarray(inputs["qry_b"], dtype=np.float32)
    key_w = np.asarray(inputs["key_w"], dtype=np.float32)
    key_b = np.asarray(inputs["key_b"], dtype=np.float32)

    from concourse import bass_utils

    nc = _get_nc()
    in_maps = make_in_maps(in_feats, feat_map, qry_w, qry_b, key_b, key_w)
    trace = os.environ.get("SEG_KERNEL_TRACE", "0") == "1"
    res = bass_utils.run_bass_kernel_spmd(
        nc, in_maps, core_ids=list(range(N_CORES)), trace=trace
    )
    _CACHE["last_result"] = res

    out = np.empty((BATCH * N_PER, FH, FW), dtype=np.float32)
    for c in range(N_CORES):
        b, h = divmod(c, 2)
        raw = res.results[c]["out"].astype(np.float32)        # [128, 3*HW]
        shard = np.empty((N_PER, HW), dtype=np.float32)
        for j, (n0, m) in enumerate(N_CHUNKS):
            shard[n0:n0 + m] = raw[0:m, j * HW:(j + 1) * HW]
        out[b * N_PER:(b + 1) * N_PER, h * HHALF:(h + 1) * HHALF, :] = (
            shard.reshape(N_PER, HHALF, FW)
        )
    return out


# revision 3
# speedup vs baseline: 1.1187x; 1.1187x over previous
"""Trainium2 Bass kernel for BaseSegHead (dynamic 1x1-conv seg logits).

Computes, for full inputs:
    qry_feats = in_feats @ qry_w.T + qry_b                  [1200, 32]
    key_map   = einsum('oc,bchw->bohw', key_w, feat_map) + key_b
    logits    = einsum('bnc,bchw->bnhw', qry_feats.reshape(4,300,32), key_map)
    out       = logits.reshape(1200, 160, 160)

Sharding: 8 cores = 4 batch images x 2 spatial (H) halves. Core c handles
batch b = c//2, rows h*80:(h+1)*80. Each core reads feat_map[b,:,rows,:],
its 300 queries, and writes a [300, 80*160] output shard -- no cross-core
communication and no duplicated feat_map reads.

Precision: matmul operands are shipped/produced as fp16 (full-rate on the
PE array; halves DMA bytes); accumulation stays fp32 in PSUM. The fp32
logits are rounded to fp16 for the output DMA and upcast on the host.

v3 layout (trace-driven): the kernel is HBM-bound (~14.4 MB of traffic).
All data DMAs ride the sync HW-DGE ring in readiness order.  feat_map is
host-packed so each of 7 input triggers delivers one 2048-column block
with BOTH channel halves (8 KB/partition rows): the first key quad can
start ~1 us after the first block lands, and quad k's operands arrive
while quad k-1's output drains.  Output is staged in three full-row SBUF
buffers [*, 12800]; 9 third-granularity out DMAs (8-9 KB rows) fire as
their drains complete.  PSUM drains are 2-bank [*, 1024] copies strictly
alternating scalar/vector in readiness order (key-quad bias-activations
take scalar slots in the same rotation) so neither engine head-of-line
blocks the other stage.

TensorE array tiling: the key projection (M=32) runs 4-way column-tiled
into one PSUM bank per quad of hw-tiles; one bias-activation drains four
tiles. The main einsum (K=32) runs 4-way row-tiled: hw-tile t keeps its
q and key_map operands on SBUF partitions 32*(t%4), so consecutive tiles
issue to distinct PE row-groups and overlap on the array.
"""

import os
import sys

sys.path.insert(0, "/opt/trn_rl_repo")
os.environ.setdefault("MYCRO_LOCAL_CACHE", "1")

import numpy as np

BATCH = 4
N_PER = 300
IN_DIM = 256
KEY_DIM = 32
FH = FW = 160
HHALF = FH // 2            # 80 rows per core
HW = HHALF * FW            # 12800 spatial positions per core
N_CORES = 8

MMN = 512                  # matmul moving free size (one fp32 PSUM bank)
N_T = HW // MMN            # 25 hw-tiles
N_BLK = 6                  # six full 2048-col blocks (quads) + one 512 tail
BLKW = 4 * MMN             # 2048 feat columns per block
# out-DMA thirds: col ranges per chunk (last absorbs the tail tile)
THIRDS = ((0, 4096), (4096, 8192), (8192, HW))
N_CHUNKS = ((0, 128), (128, 128), (256, 44))   # query-row chunks (300 rows)
CPACK_W = 728              # fp16: qry_wT (64) + in_featsT (600) + key_wT (64)
FPW = 2 * HW               # featP width: d0|d1 interleaved per block

_CACHE = {}


def build_nc():
    import concourse.bass as bass
    import concourse.bacc as bacc
    import concourse.mybir as mybir
    from concourse import tile

    f32 = mybir.dt.float32
    f16 = mybir.dt.float16
    Ident = mybir.ActivationFunctionType.Identity

    nc = bacc.Bacc("TRN2", target_bir_lowering=False, debug=False)

    featP = nc.dram_tensor("featP", [128, FPW], f16, kind="ExternalInput")
    cpack = nc.dram_tensor("cpack", [128, CPACK_W], f16, kind="ExternalInput")
    bpack = nc.dram_tensor("bpack", [128, 2], f32, kind="ExternalInput")
    out = nc.dram_tensor("out", [128, 3 * HW], f16, kind="ExternalOutput")

    with tile.TileContext(nc) as tc:
        with (
            tc.tile_pool(name="const", bufs=1) as cpool,
            tc.tile_pool(name="fpool", bufs=1) as fpool,
            tc.tile_pool(name="opool", bufs=1) as opool,
            tc.tile_pool(name="kmap", bufs=1) as kpool,
            tc.tile_pool(name="ps_main", bufs=3, space=bass.MemorySpace.PSUM) as ps_main,
            tc.tile_pool(name="ps_small", bufs=2, space=bass.MemorySpace.PSUM) as ps_small,
        ):
            # --- DMA ring head: consts, then the 7 paired feat blocks ----
            ct = cpool.tile([128, CPACK_W], f16, name="ct")
            nc.sync.dma_start(ct[:], cpack[:])
            bt = cpool.tile([128, 2], f32, name="bt")
            nc.sync.dma_start(bt[:], bpack[:])
            qw = (ct[:, 0:32], ct[:, 32:64])
            inT = (ct[:, 64:364], ct[:, 364:664])
            kw = (ct[:, 664:696], ct[:, 696:728])
            qb = bt[:, 0:1]        # qry_b replicated in all four bands
            kb = bt[:, 1:2]        # key_b replicated in all four bands

            # featP block k: cols [2k*BLKW, (2k+2)*BLKW) = d0 block | d1 block
            fp = fpool.tile([128, FPW], f16, name="fp")
            for k in range(N_BLK):
                nc.sync.dma_start(
                    fp[:, 2 * k * BLKW:2 * (k + 1) * BLKW],
                    featP[:, 2 * k * BLKW:2 * (k + 1) * BLKW],
                )
            # tail block: one 512-col tile per half
            nc.sync.dma_start(fp[:, 2 * N_BLK * BLKW:FPW],
                              featP[:, 2 * N_BLK * BLKW:FPW])

            def feat(d, t):
                # hw-tile t, channel half d -> fp column range
                k = t // 4
                if k < N_BLK:
                    c0 = 2 * k * BLKW + d * BLKW + (t % 4) * MMN
                else:
                    c0 = 2 * N_BLK * BLKW + d * MMN
                return fp[:, c0:c0 + MMN]

            # --- qry projection, 4-way column-tiled (4 band copies) -------
            qp = ps_small.tile([128, MMN], f32, name="qp", tag="kp")
            for b in range(4):
                for d in range(2):
                    nc.tensor.matmul(
                        qp[32 * b:32 * b + 32, 0:N_PER],
                        qw[d],
                        inT[d],
                        start=(d == 0),
                        stop=(d == 1),
                        tile_position=(0, 32 * b),
                    )
            q_sb = cpool.tile([128, N_PER], f16, name="q_sb")
            nc.scalar.activation(q_sb[:], qp[:, 0:N_PER], Ident, bias=qb)

            # --- key_map: 4-way column-tiled, banded layout ---------------
            # hw-tile t lives on SBUF partitions 32*(t%4), columns
            # (t//4)*512; one [128,512] PSUM bank holds a whole quad and is
            # drained by a single bias-activation.
            key_map = kpool.tile([128, 7 * MMN], f16, name="key_map")

            def key_quad(k):
                kp = ps_small.tile([128, MMN], f32, name=f"kp_{k}", tag="kp")
                nb = min(4, N_T - 4 * k)
                for b in range(nb):
                    for d in range(2):
                        nc.tensor.matmul(
                            kp[32 * b:32 * b + 32, :],
                            kw[d],
                            feat(d, 4 * k + b),
                            start=(d == 0),
                            stop=(d == 1),
                            tile_position=(0, 32 * b),
                        )
                p = 32 * nb
                nc.scalar.activation(
                    key_map[0:p, k * MMN:(k + 1) * MMN], kp[0:p, :], Ident,
                    bias=kb[0:p, :],
                )

            # --- output row-buffers: one [*, 12800] tile per query chunk --
            OB = [opool.tile([128, HW], f16, name=f"ob_{j}") for j in range(3)]

            # --- main einsum: 4-way row-tiled over band b = t%4 -----------
            # Pairs of hw-tiles share a 2-bank PSUM tile so one copy drains
            # both; drains alternate vector/scalar in readiness order.
            cp = 0

            def drain(dst, src):
                nonlocal cp
                if cp % 2 == 0:
                    nc.vector.tensor_copy(dst, src)
                else:
                    nc.scalar.copy(dst, src)
                cp += 1

            def main_pair(p):
                t0 = 2 * p
                for j, (n0, m) in enumerate(N_CHUNKS):
                    mp = ps_main.tile([128, 2 * MMN], f32, name=f"mp_{p}_{n0}", tag="mp")
                    for i, t in enumerate((t0, t0 + 1)):
                        b = t % 4
                        kcol = (t // 4) * MMN
                        nc.tensor.matmul(
                            mp[:m, i * MMN:(i + 1) * MMN],
                            q_sb[32 * b:32 * b + 32, n0:n0 + m],
                            key_map[32 * b:32 * b + 32, kcol:kcol + MMN],
                            tile_position=(32 * b, 0),
                        )
                    drain(OB[j][:m, t0 * MMN:(t0 + 2) * MMN], mp[:m, :])

            def main_tail():
                # hw-tile 24: band 0, key_map quad 6
                for j, (n0, m) in enumerate(N_CHUNKS):
                    mp = ps_small.tile([128, MMN], f32, name=f"mt_{n0}", tag="kp")
                    nc.tensor.matmul(
                        mp[:m, :],
                        q_sb[0:32, n0:n0 + m],
                        key_map[0:32, 6 * MMN:7 * MMN],
                        tile_position=(0, 0),
                    )
                    drain(OB[j][:m, 24 * MMN:25 * MMN], mp[:m, :])

            # Interleave: each key quad feeds two main pairs.
            for k in range(N_BLK):
                key_quad(k)
                main_pair(2 * k)
                main_pair(2 * k + 1)
            key_quad(6)
            main_tail()

            # --- out DMAs: third-granularity, readiness order ------------
            for (c0, c1) in THIRDS:
                for j, (n0, m) in enumerate(N_CHUNKS):
                    nc.sync.dma_start(
                        out[0:m, j * HW + c0:j * HW + c1], OB[j][0:m, c0:c1]
                    )

    nc.compile()
    return nc


def _get_nc():
    if "nc" not in _CACHE:
        _CACHE["nc"] = build_nc()
    return _CACHE["nc"]


def make_in_maps(in_feats, feat_map, qry_w, qry_b, key_b, key_w):
    qwT = qry_w.T.astype(np.float16)                          # [256, 32]
    kwT = key_w.T.astype(np.float16)                          # [256, 32]
    bpack = np.zeros((128, 2), np.float32)
    bpack[:, 0] = np.tile(qry_b, 4)
    bpack[:, 1] = np.tile(key_b, 4)
    in_maps = []
    for c in range(N_CORES):
        b, h = divmod(c, 2)
        ifT = in_feats[b * N_PER:(b + 1) * N_PER].T.astype(np.float16)
        cpack = np.zeros((128, CPACK_W), np.float16)
        cpack[:, 0:32] = qwT[0:128]
        cpack[:, 32:64] = qwT[128:256]
        cpack[:, 64:364] = ifT[0:128]
        cpack[:, 364:664] = ifT[128:256]
        cpack[:, 664:696] = kwT[0:128]
        cpack[:, 696:728] = kwT[128:256]
        feat16 = np.ascontiguousarray(
            feat_map[b, :, h * HHALF:(h + 1) * HHALF, :]
        ).reshape(IN_DIM, HW).astype(np.float16)
        # featP: block k holds cols [2k*BLKW,(2k+2)*BLKW) = d0 cols | d1 cols
        featP = np.empty((128, FPW), np.float16)
        for k in range(N_BLK + 1):
            w = BLKW if k < N_BLK else MMN
            c0 = k * BLKW
            for d in range(2):
                featP[:, 2 * c0 + d * w:2 * c0 + (d + 1) * w] = (
                    feat16[d * 128:(d + 1) * 128, c0:c0 + w]
                )
        in_maps.append({
            "featP": featP,
            "cpack": cpack,
            "bpack": bpack,
        })
    return in_maps


def kernel(**inputs):
    in_feats = np.asarray(inputs["in_feats"], dtype=np.float32)
    feat_map = np.asarray(inputs["feat_map"], dtype=np.float32)
    qry_w = np.asarray(inputs["qry_w"], dtype=np.float32)
    qry_b = np.asarray(inputs["qry_b"], dtype=np.float32)
    key_w = np.asarray(inputs["key_w"], dtype=np.float32)
    key_b = np.asarray(inputs["key_b"], dtype=np.float32)

    from concourse import bass_utils

    nc = _get_nc()
    in_maps = make_in_maps(in_feats, feat_map, qry_w, qry_b, key_b, key_w)
    trace = os.environ.get("SEG_KERNEL_TRACE", "0") == "1"
    res = bass_utils.run_bass_kernel_spmd(
        nc, in_maps, core_ids=list(range(N_CORES)), trace=trace
    )
    _CACHE["last_result"] = res

    out = np.empty((BATCH * N_PER, FH, FW), dtype=np.float32)
    for c in range(N_CORES):
        b, h = divmod(c, 2)
        raw = res.results[c]["out"].astype(np.float32)        # [128, 3*HW]
        shard = np.empty((N_PER, HW), dtype=np.float32)
        for j, (n0, m) in enumerate(N_CHUNKS):
            shard[n0:n0 + m] = raw[0:m, j * HW:(j + 1) * HW]
        out[b * N_PER:(b + 1) * N_PER, h * HHALF:(h + 1) * HHALF, :] = (
            shard.reshape(N_PER, HHALF, FW)
        )
    return out
